# revision 32
# baseline (speedup 1.0000x reference)
"""Joint attention layer on 8 trn2 NeuronCores (query-sharded, SPMD).

Math (reference):
    Q = img @ Wq.T ; K = text @ Wk.T ; S = Q @ K.T        [N, N]
    attn = softmax(S, axis=1) / sqrt(D)
    out_img = attn @ img ; out_text = attn @ text

Per-core plan (core c owns query rows m in [c*1024, (c+1)*1024)):
    H[j,i]  = sum_d Wq[d,j] Wk[d,i]              (host, weight-only prep)
    G[i,m]  = sum_j H[j,i] imgT[j,m]             (absorbs both projections)
    S^T[n,m] = sum_i text[n,i] G[i,m]            (keys on partitions)
    P^T = exp(S^T)  (no max subtraction needed: |S| <~ 55 << 88)
    O[m,:] = sum_n P^T[n,m] * [img|text][n,:]    (PSUM accum over all n)
    ptacc[n128,m] = sum_ch P^T chunks            (DVE, elementwise)
    rowsum[m] (x16) = ptacc.T @ ones16           (4 tiny K=128 bf16 matmuls
                                                  per m-block; the 16 folds
                                                  1/sqrt(D) into the recip)
    out[m,:] = O[m,:] * recip[m]                 (DVE/ACT split, bf16 out)

Precision: S-chain (H,imgT,G,textT) in fp16 (values are O(1)); P^T and
the O matmul in bf16 (exp values reach ~e^55, beyond fp16 range); all
accumulation in fp32 (PSUM / DVE); output rounded to bf16.  fp32
matmuls are avoided everywhere (LOW/HIGH double pass + array drain).

PE budget is the roofline here (768 matmuls of 512 fp16/bf16 columns,
1 col/cycle @2.4GHz = ~166us): the rowsum is kept OFF the PE (DVE
accumulates the exp chunks; one 4-column matmul per m-block extracts it
directly in query-partition layout), startup DMAs are ordered so the
first matmuls' operands land first (H on the scalar queue; imgT, textT
group 0, then rhs pairs + textT prefetches in consumption order on the
sync queue), and a burst of dummy matmuls during the initial DMA wait
warms the PE HAM clock-gate so the real stream starts at 2.4GHz.

Host passes text transposed (fp16), img/text packed+interleaved (bf16)
so one 2KB-row DMA feeds two O-phase key chunks, and H precomputed; the
kernel never transposes on device. No collectives: outputs are disjoint
row slabs concatenated on the host.
"""

import numpy as np
import ml_dtypes
from contextlib import ExitStack

import concourse.bass as bass
import concourse.tile as tile
from concourse import bacc, mybir
from concourse.bass_utils import run_bass_kernel_spmd

F32 = mybir.dt.float32
F16 = mybir.dt.float16
BF16 = mybir.dt.bfloat16
P = 128          # partitions
D = 256          # hidden dim
N = 8192         # sequence length
N_CORES = 8
SLAB = N // N_CORES          # 1024 query rows per core
MB = 2                       # m-blocks per core
MBS = SLAB // MB             # 512 queries per m-block
NSUB = MBS // P              # 4 psum subtiles per m-block
NCH = N // P                 # 64 key chunks of 128
TTG = 8                      # textT column-group tiles
TTW = N // TTG               # 1024 cols per group
CHG = TTW // P               # chunks per textT group (8)
PIPE = 3                     # S-stage lookahead (chunks)
NWARM = 16                   # HAM warmup matmuls during initial DMA wait

_CACHE = {}


def _build_nc():
    nc = bacc.Bacc("TRN2", target_bir_lowering=False, debug=False,
                   num_devices=N_CORES)

    # it2: [img|text] bf16 rows, two key-chunks packed per 128-row block
    # (see kernel() for the host-side packing) -> one 2KB-row DMA feeds
    # two O-phase chunks.
    it2_d = nc.dram_tensor("it2_bf16", [N // 2, 4 * D], BF16,
                           kind="ExternalInput").ap()
    textT_d = nc.dram_tensor("textT_f16", [D, N], F16, kind="ExternalInput").ap()
    imgT_d = nc.dram_tensor("imgT_f16", [D, SLAB], F16, kind="ExternalInput").ap()
    # H = Wq.T @ Wk precomputed on host (weight-only prep, like the
    # transposes): kills 4 setup matmuls and halves the critical-path
    # weight DMA.
    h_d = nc.dram_tensor("H_f16", [D, D], F16, kind="ExternalInput").ap()
    out_d = nc.dram_tensor("out", [SLAB, 2 * D], BF16, kind="ExternalOutput").ap()

    with tile.TileContext(nc) as tc:
        with ExitStack() as ctx:
            const = ctx.enter_context(tc.tile_pool(name="const", bufs=1))

            # HAM warmup scratch (zeros; results never read)
            warm_w = const.tile([P, P], BF16, name="warm_w")
            warm_x = const.tile([P, 256], BF16, name="warm_x")
            nc.vector.memset(warm_w[:], 0.0)
            nc.vector.memset(warm_x[:], 0.0)

            # Startup DMAs split over queues so the critical chain
            # (H + imgT -> G, tt[0] -> S) isn't serialized behind the
            # bulk input flood:
            #   scalar q: H (tiny)
            #   sync q:   imgT, tt[0], then rhs pairs + tt-group
            #             prefetches in consumption order, out at the end
            h_sb = [const.tile([P, D], F16, name=f"h{jt}") for jt in range(2)]
            for t in range(2):
                nc.scalar.dma_start(h_sb[t][:], h_d[t * P:(t + 1) * P, :])

            imgT_sb = [const.tile([P, SLAB], F16, name=f"imgT{t}") for t in range(2)]
            for t in range(2):
                nc.sync.dma_start(imgT_sb[t][:], imgT_d[t * P:(t + 1) * P, :])

            tt_sb = [[const.tile([P, TTW], F16, name=f"tt{it}_{g}")
                      for g in range(TTG)] for it in range(2)]
            for it in range(2):
                nc.sync.dma_start(tt_sb[it][0][:],
                                  textT_d[it * P:(it + 1) * P, 0:TTW])

            ones16 = const.tile([P, 1], BF16, name="ones16")
            nc.vector.memset(ones16[:], 16.0)   # folds 1/sqrt(D) into recip

            g_sb = [const.tile([P, SLAB], F16, name=f"g{it}") for it in range(2)]

            # ---- warmup + setup: G[i,m] = sum_j H[j,i] imgT[j,m] ----
            with tc.tile_pool(name="psetup", bufs=2, space="PSUM") as psetup:
                for i in range(NWARM):
                    wp = psetup.tile([P, 256], F32, tag="warm", name=f"warm{i}")
                    nc.tensor.matmul(wp[:], lhsT=warm_w[:], rhs=warm_x[:],
                                     start=True, stop=True)
                for hh in range(2):
                    for it in range(2):
                        gp = psetup.tile([P, MBS], F32, tag="g", name=f"gp{it}_{hh}")
                        for jt in range(2):
                            nc.tensor.matmul(
                                gp[:],
                                lhsT=h_sb[jt][:, it * P:(it + 1) * P],
                                rhs=imgT_sb[jt][:, hh * MBS:(hh + 1) * MBS],
                                start=(jt == 0), stop=(jt == 1))
                        nc.vector.tensor_copy(g_sb[it][:, hh * MBS:(hh + 1) * MBS],
                                              gp[:])

            # ---- main pools ----
            o_pool = ctx.enter_context(tc.tile_pool(name="opool", bufs=4, space="PSUM"))
            s_pool = ctx.enter_context(tc.tile_pool(name="spool", bufs=PIPE + 1, space="PSUM"))
            rhs_pool = ctx.enter_context(tc.tile_pool(name="rhs", bufs=NCH // 2))
            pt_pool = ctx.enter_context(tc.tile_pool(name="pt", bufs=PIPE + 5))
            eout_pool = ctx.enter_context(tc.tile_pool(name="eout", bufs=4))
            rec_pool = ctx.enter_context(tc.tile_pool(name="rec", bufs=3 * MB))

            rhs_tiles = {}

            def s_mm(mb, ch, it, sp):
                g, coff = divmod(ch, CHG)
                coff *= P
                nc.tensor.matmul(
                    sp[:],
                    lhsT=tt_sb[it][g][:, coff:coff + P],
                    rhs=g_sb[it][:, mb * MBS:(mb + 1) * MBS],
                    start=(it == 0), stop=(it == 1))

            def s_act(mb, ch, sp):
                pt = pt_pool.tile([P, MBS], BF16, tag="pt", name=f"pt{mb}_{ch}")
                nc.scalar.activation(pt[:], sp[:],
                                     mybir.ActivationFunctionType.Exp)
                return pt

            for mb in range(MB):
                o_ps = [o_pool.tile([P, 2 * D], F32, tag="o", name=f"o{mb}_{i}")
                        for i in range(NSUB)]
                ptacc = rec_pool.tile([P, MBS], F32, tag="ptacc",
                                      name=f"ptacc{mb}")

                pts = {}
                for ch in range(PIPE):
                    sp = s_pool.tile([P, MBS], F32, tag="s", name=f"s{mb}_{ch}")
                    s_mm(mb, ch, 0, sp)
                    s_mm(mb, ch, 1, sp)
                    pts[ch] = s_act(mb, ch, sp)

                for ch in range(NCH):
                    nxt = ch + PIPE
                    sp_n = None
                    if nxt < NCH:
                        sp_n = s_pool.tile([P, MBS], F32, tag="s",
                                           name=f"s{mb}_{nxt}")

                    if mb == 0:
                        if ch % 2 == 0:
                            r2 = rhs_pool.tile([P, 4 * D], BF16, tag="rhs",
                                               name=f"rhs{ch // 2}")
                            nc.sync.dma_start(
                                r2[:], it2_d[(ch // 2) * P:(ch // 2 + 1) * P, :])
                            rhs_tiles[ch] = r2[:, 0:2 * D]
                            rhs_tiles[ch + 1] = r2[:, 2 * D:4 * D]
                        # prefetch the next textT group right when the
                        # previous one starts being consumed
                        if ch % CHG == 0 and ch // CHG + 1 < TTG:
                            g_nxt = ch // CHG + 1
                            for it in range(2):
                                nc.sync.dma_start(
                                    tt_sb[it][g_nxt][:],
                                    textT_d[it * P:(it + 1) * P,
                                            g_nxt * TTW:(g_nxt + 1) * TTW])
                        rhs = rhs_tiles[ch]
                    else:
                        rhs = rhs_tiles[ch]

                    pt = pts.pop(ch)
                    # running elementwise sum of exp chunks (DVE) — feeds
                    # the rowsum extraction, keeping it off the PE
                    if ch == 0:
                        nc.vector.tensor_copy(ptacc[:], pt[:])
                    else:
                        nc.vector.tensor_add(ptacc[:], ptacc[:], pt[:])

                    first, last = (ch == 0), (ch == NCH - 1)

                    def o_mm(sub):
                        nc.tensor.matmul(o_ps[sub][:],
                                         lhsT=pt[:, sub * P:(sub + 1) * P],
                                         rhs=rhs, start=first, stop=last)

                    # Interleave fresh-weight S MMs between pt-weight O MMs
                    # so every LDWEIGHTS hides under a full 512-col stream.
                    if sp_n is not None:
                        s_mm(mb, nxt, 0, sp_n)
                    o_mm(0)
                    if sp_n is not None:
                        s_mm(mb, nxt, 1, sp_n)
                        pts[nxt] = s_act(mb, nxt, sp_n)
                    o_mm(1)
                    if last:
                        # rowsum*16 per query sub-block: ptacc.T @ ones16.
                        # bf16 (single rounding of the final sums) so the
                        # PE stays in 16-bit mode — fp32 matmuls here cost
                        # a LOW/HIGH double pass + array drain. Placed
                        # between O MMs so the recip chain overlaps them.
                        pt16 = rec_pool.tile([P, MBS], BF16, tag="pt16",
                                             name=f"pt16_{mb}")
                        nc.vector.tensor_copy(pt16[:], ptacc[:])
                        rsq = s_pool.tile([P, NSUB], F32, tag="s",
                                          name=f"rsq{mb}")
                        for sub in range(NSUB):
                            nc.tensor.matmul(
                                rsq[:, sub:sub + 1],
                                lhsT=pt16[:, sub * P:(sub + 1) * P],
                                rhs=ones16[:],
                                start=(sub == 0), stop=(sub == NSUB - 1),
                                skip_group_check=True)
                        recip = rec_pool.tile([P, NSUB], F32, tag="recip",
                                              name=f"recip{mb}")
                        nc.vector.reciprocal(recip[:], rsq[:])
                    o_mm(2)
                    o_mm(3)

                # epilogue: out = O * (NORM/rowsum); split DVE/ACT
                for sub in range(NSUB):
                    osb = eout_pool.tile([P, 2 * D], BF16, tag="eout",
                                         name=f"eout{mb}_{sub}")
                    # mb=0: all muls on DVE so ACT stays exp-dedicated for
                    # the next m-block's prologue; last mb: split DVE/ACT
                    # to halve the tail.
                    if mb + 1 < MB or sub % 2 == 0:
                        nc.vector.tensor_scalar_mul(
                            osb[:], o_ps[sub][:], recip[:, sub:sub + 1])
                    else:
                        nc.scalar.mul(osb[:], o_ps[sub][:],
                                      recip[:, sub:sub + 1])
                    row0 = mb * MBS + sub * P
                    # split output DMAs across two queues to shorten the tail
                    if sub % 2 == 0:
                        nc.sync.dma_start(out_d[row0:row0 + P, :], osb[:])
                    else:
                        nc.scalar.dma_start(out_d[row0:row0 + P, :], osb[:])

    nc.compile()
    return nc


def kernel(img, text, Wq, Wk):
    img = np.ascontiguousarray(img, dtype=np.float32)
    text = np.ascontiguousarray(text, dtype=np.float32)

    if "nc" not in _CACHE:
        _CACHE["nc"] = _build_nc()
    nc = _CACHE["nc"]

    textT16 = np.ascontiguousarray(text.T.astype(np.float16))
    # [img|text] bf16 rows, then two key-chunks (2*128 rows) packed side by
    # side so one 2KB-row DMA feeds two O-phase chunks.
    it_bf = np.concatenate([img, text], axis=1).astype(ml_dtypes.bfloat16)
    it2 = np.ascontiguousarray(
        it_bf.reshape(N // (2 * P), 2, P, 2 * D)
        .transpose(0, 2, 1, 3).reshape(N // 2, 4 * D))
    h16 = np.ascontiguousarray(
        (np.asarray(Wq, dtype=np.float32).T
         @ np.asarray(Wk, dtype=np.float32)).astype(np.float16))

    in_maps = []
    for c in range(N_CORES):
        in_maps.append({
            "it2_bf16": it2,
            "textT_f16": textT16,
            "imgT_f16": np.ascontiguousarray(
                img[c * SLAB:(c + 1) * SLAB].T.astype(np.float16)),
            "H_f16": h16,
        })

    res = run_bass_kernel_spmd(nc, in_maps, core_ids=list(range(N_CORES)),
                               **_CACHE.get("run_kwargs", {}))
    _CACHE["last_results"] = res
    out = np.concatenate([np.asarray(res.results[c]["out"], dtype=np.float32)
                          for c in range(N_CORES)], axis=0)
    return np.ascontiguousarray(out[:, :D]), np.ascontiguousarray(out[:, D:])


if __name__ == "__main__":
    rng = np.random.default_rng(0)
    img = rng.standard_normal((N, D), dtype=np.float32)
    text = rng.standard_normal((N, D), dtype=np.float32)
    sc = 1.0 / np.sqrt(D)
    Wq = rng.uniform(-sc, sc, (D, D)).astype(np.float32)
    Wk = rng.uniform(-sc, sc, (D, D)).astype(np.float32)
    oi, ot = kernel(img, text, Wq, Wk)
    print("out_img", oi.shape, oi.dtype, "out_text", ot.shape, ot.dtype)


# revision 40
# speedup vs baseline: 1.1944x; 1.1944x over previous
"""Joint attention layer on 8 trn2 NeuronCores (query-sharded, SPMD).

Math (reference):
    Q = img @ Wq.T ; K = text @ Wk.T ; S = Q @ K.T        [N, N]
    attn = softmax(S, axis=1) / sqrt(D)
    out_img = attn @ img ; out_text = attn @ text

Per-core plan (core c owns query rows m in [c*1024, (c+1)*1024)):
    H[j,i]  = sum_d Wq[d,j] Wk[d,i]              (host, weight-only prep)
    G[i,m]  = sum_j H[j,i] imgT[j,m]             (absorbs both projections)
    S^T[n,m] = sum_i text[n,i] G[i,m]            (keys on partitions)
    P^T = exp(S^T)  (no max subtraction needed: |S| <~ 55 << 88)
    O[m,:] = sum_n P^T[n,m] * [img|text][n,:]    (PSUM accum over all n)
    ptacc[n128,m] = sum_ch P^T chunks            (DVE, elementwise)
    rowsum[m] (x16) = ptacc.T @ ones16           (4 tiny K=128 bf16 matmuls
                                                  per m-block; the 16 folds
                                                  1/sqrt(D) into the recip)
    out[m,:] = O[m,:] * recip[m]                 (DVE/ACT split, bf16 out)

Precision: S-chain (H,imgT,G,textT) in fp16 (values are O(1)); P^T and
the O matmul in bf16 (exp values reach ~e^55, beyond fp16 range); all
accumulation in fp32 (PSUM / DVE); output rounded to bf16.  fp32
matmuls are avoided everywhere (LOW/HIGH double pass + array drain).

PE budget is the roofline here (768 matmuls of 512 fp16/bf16 columns,
1 col/cycle @2.4GHz = ~166us): the rowsum is kept OFF the PE (DVE
accumulates the exp chunks; one 4-column matmul per m-block extracts it
directly in query-partition layout), startup DMAs are ordered so the
first matmuls' operands land first (H on the scalar queue; imgT, textT
group 0, then rhs pairs + textT prefetches in consumption order on the
sync queue), and a burst of dummy matmuls during the initial DMA wait
warms the PE HAM clock-gate so the real stream starts at 2.4GHz.

Host passes text transposed (fp16), img/text packed+interleaved (bf16)
so one 2KB-row DMA feeds two O-phase key chunks, and H precomputed; the
kernel never transposes on device. No collectives: outputs are disjoint
row slabs concatenated on the host.
"""

import numpy as np
import ml_dtypes
from contextlib import ExitStack

import concourse.bass as bass
import concourse.tile as tile
from concourse import bacc, mybir
from concourse.bass_utils import run_bass_kernel_spmd

F32 = mybir.dt.float32
F16 = mybir.dt.float16
BF16 = mybir.dt.bfloat16
P = 128          # partitions
D = 256          # hidden dim
N = 8192         # sequence length
N_CORES = 8
SLAB = N // N_CORES          # 1024 query rows per core
MB = 2                       # m-blocks per core
MBS = SLAB // MB             # 512 queries per m-block
NSUB = MBS // P              # 4 psum subtiles per m-block
NCH = N // P                 # 64 key chunks of 128
TTG = 8                      # textT column-group tiles
TTW = N // TTG               # 1024 cols per group
CHG = TTW // P               # chunks per textT group (8)
PIPE = 3                     # S-stage lookahead (chunks)
NWARM = 16                   # HAM warmup matmuls during initial DMA wait

_CACHE = {}


def _build_nc():
    nc = bacc.Bacc("TRN2", target_bir_lowering=False, debug=False,
                   num_devices=N_CORES)

    # it2: [img|text] bf16 rows, two key-chunks packed per 128-row block
    # (see kernel() for the host-side packing) -> one 2KB-row DMA feeds
    # two O-phase chunks.
    it2_d = nc.dram_tensor("it2_bf16", [N // 2, 4 * D], BF16,
                           kind="ExternalInput").ap()
    # textT with the two d-halves (it) of each column group packed side by
    # side: one tile + one DMA per group instead of two (halves the PE's
    # first-touch semaphore waits and the sync-queue issue slots)
    ttp_d = nc.dram_tensor("ttp_f16", [P, 2 * N], F16, kind="ExternalInput").ap()
    imgT_d = nc.dram_tensor("imgT_f16", [D, SLAB], F16, kind="ExternalInput").ap()
    # H = Wq.T @ Wk precomputed on host (weight-only prep, like the
    # transposes): kills 4 setup matmuls and halves the critical-path
    # weight DMA.
    h_d = nc.dram_tensor("H_f16", [D, D], F16, kind="ExternalInput").ap()
    # output column-packed: [128 partitions, 8 query-blocks x 512] so a
    # sub-block pair ships as one 2KB-row DMA; host unpacks
    out_d = nc.dram_tensor("out", [P, (SLAB // P) * 2 * D], BF16,
                           kind="ExternalOutput").ap()

    with tile.TileContext(nc) as tc:
        with ExitStack() as ctx:
            const = ctx.enter_context(tc.tile_pool(name="const", bufs=1))

            # HAM warmup scratch (zeros; results never read)
            warm_w = const.tile([P, P], BF16, name="warm_w")
            warm_x = const.tile([P, 256], BF16, name="warm_x")
            nc.vector.memset(warm_w[:], 0.0)
            nc.vector.memset(warm_x[:], 0.0)

            # Startup DMAs split over queues so the critical chain
            # (H + imgT -> G, tt[0] -> S) isn't serialized behind the
            # bulk input flood:
            #   scalar q: H (tiny)
            #   sync q:   imgT, tt[0], then rhs pairs + tt-group
            #             prefetches in consumption order, out at the end
            h_sb = [const.tile([P, D], F16, name=f"h{jt}") for jt in range(2)]
            for t in range(2):
                nc.scalar.dma_start(h_sb[t][:], h_d[t * P:(t + 1) * P, :])

            imgT_sb = [const.tile([P, SLAB], F16, name=f"imgT{t}") for t in range(2)]
            for t in range(2):
                nc.sync.dma_start(imgT_sb[t][:], imgT_d[t * P:(t + 1) * P, :])

            tt_sb = [const.tile([P, 2 * TTW], F16, name=f"tt{g}")
                     for g in range(TTG)]
            nc.sync.dma_start(tt_sb[0][:], ttp_d[:, 0:2 * TTW])

            ones16 = const.tile([P, 1], BF16, name="ones16")
            nc.vector.memset(ones16[:], 16.0)   # folds 1/sqrt(D) into recip

            g_sb = [const.tile([P, SLAB], F16, name=f"g{it}") for it in range(2)]

            # ---- warmup + setup: G[i,m] = sum_j H[j,i] imgT[j,m] ----
            with tc.tile_pool(name="psetup", bufs=2, space="PSUM") as psetup:
                for i in range(NWARM):
                    wp = psetup.tile([P, 256], F32, tag="warm", name=f"warm{i}")
                    nc.tensor.matmul(wp[:], lhsT=warm_w[:], rhs=warm_x[:],
                                     start=True, stop=True)
                for hh in range(2):
                    for it in range(2):
                        gp = psetup.tile([P, MBS], F32, tag="g", name=f"gp{it}_{hh}")
                        for jt in range(2):
                            nc.tensor.matmul(
                                gp[:],
                                lhsT=h_sb[jt][:, it * P:(it + 1) * P],
                                rhs=imgT_sb[jt][:, hh * MBS:(hh + 1) * MBS],
                                start=(jt == 0), stop=(jt == 1))
                        nc.vector.tensor_copy(g_sb[it][:, hh * MBS:(hh + 1) * MBS],
                                              gp[:])

            # ---- main pools ----
            o_pool = ctx.enter_context(tc.tile_pool(name="opool", bufs=4, space="PSUM"))
            s_pool = ctx.enter_context(tc.tile_pool(name="spool", bufs=PIPE + 1, space="PSUM"))
            rhs_pool = ctx.enter_context(tc.tile_pool(name="rhs", bufs=NCH // 2))
            pt_pool = ctx.enter_context(tc.tile_pool(name="pt", bufs=PIPE + 5))
            eout_pool = ctx.enter_context(tc.tile_pool(name="eout", bufs=4))
            rec_pool = ctx.enter_context(tc.tile_pool(name="rec", bufs=3 * MB))

            rhs_tiles = {}

            def s_mm(mb, ch, it, sp):
                g, coff = divmod(ch, CHG)
                coff = it * TTW + coff * P
                nc.tensor.matmul(
                    sp[:],
                    lhsT=tt_sb[g][:, coff:coff + P],
                    rhs=g_sb[it][:, mb * MBS:(mb + 1) * MBS],
                    start=(it == 0), stop=(it == 1))

            def s_act(mb, ch, sp):
                pt = pt_pool.tile([P, MBS], BF16, tag="pt", name=f"pt{mb}_{ch}")
                nc.scalar.activation(pt[:], sp[:],
                                     mybir.ActivationFunctionType.Exp)
                return pt

            for mb in range(MB):
                o_ps = [o_pool.tile([P, 2 * D], F32, tag="o", name=f"o{mb}_{i}")
                        for i in range(NSUB)]
                ptacc = rec_pool.tile([P, MBS], F32, tag="ptacc",
                                      name=f"ptacc{mb}")

                pts = {}
                for ch in range(PIPE):
                    sp = s_pool.tile([P, MBS], F32, tag="s", name=f"s{mb}_{ch}")
                    s_mm(mb, ch, 0, sp)
                    s_mm(mb, ch, 1, sp)
                    pts[ch] = s_act(mb, ch, sp)

                for ch in range(NCH):
                    nxt = ch + PIPE
                    sp_n = None
                    if nxt < NCH:
                        sp_n = s_pool.tile([P, MBS], F32, tag="s",
                                           name=f"s{mb}_{nxt}")

                    if mb == 0:
                        if ch % 2 == 0:
                            r2 = rhs_pool.tile([P, 4 * D], BF16, tag="rhs",
                                               name=f"rhs{ch // 2}")
                            nc.sync.dma_start(
                                r2[:], it2_d[(ch // 2) * P:(ch // 2 + 1) * P, :])
                            rhs_tiles[ch] = r2[:, 0:2 * D]
                            rhs_tiles[ch + 1] = r2[:, 2 * D:4 * D]
                        # prefetch the next textT group right when the
                        # previous one starts being consumed
                        if ch % CHG == 0 and ch // CHG + 1 < TTG:
                            g_nxt = ch // CHG + 1
                            nc.sync.dma_start(
                                tt_sb[g_nxt][:],
                                ttp_d[:, g_nxt * 2 * TTW:(g_nxt + 1) * 2 * TTW])
                        rhs = rhs_tiles[ch]
                    else:
                        rhs = rhs_tiles[ch]

                    pt = pts.pop(ch)
                    # running elementwise sum of exp chunks (DVE) — feeds
                    # the rowsum extraction, keeping it off the PE
                    if ch == 0:
                        nc.vector.tensor_copy(ptacc[:], pt[:])
                    else:
                        nc.vector.tensor_add(ptacc[:], ptacc[:], pt[:])

                    first, last = (ch == 0), (ch == NCH - 1)

                    def o_mm(sub):
                        nc.tensor.matmul(o_ps[sub][:],
                                         lhsT=pt[:, sub * P:(sub + 1) * P],
                                         rhs=rhs, start=first, stop=last)

                    # Interleave fresh-weight S MMs between pt-weight O MMs
                    # so every LDWEIGHTS hides under a full 512-col stream.
                    if sp_n is not None:
                        s_mm(mb, nxt, 0, sp_n)
                    o_mm(0)
                    if sp_n is not None:
                        s_mm(mb, nxt, 1, sp_n)
                        pts[nxt] = s_act(mb, nxt, sp_n)
                    o_mm(1)
                    if last:
                        # rowsum*16 per query sub-block: ptacc.T @ ones16.
                        # bf16 (single rounding of the final sums) so the
                        # PE stays in 16-bit mode — fp32 matmuls here cost
                        # a LOW/HIGH double pass + array drain. Placed
                        # between O MMs so the recip chain overlaps them.
                        pt16 = rec_pool.tile([P, MBS], BF16, tag="pt16",
                                             name=f"pt16_{mb}")
                        nc.vector.tensor_copy(pt16[:], ptacc[:])
                        rsq = s_pool.tile([P, NSUB], F32, tag="s",
                                          name=f"rsq{mb}")
                        for sub in range(NSUB):
                            nc.tensor.matmul(
                                rsq[:, sub:sub + 1],
                                lhsT=pt16[:, sub * P:(sub + 1) * P],
                                rhs=ones16[:],
                                start=(sub == 0), stop=(sub == NSUB - 1),
                                skip_group_check=True)
                        recip = rec_pool.tile([P, NSUB], F32, tag="recip",
                                              name=f"recip{mb}")
                        nc.vector.reciprocal(recip[:], rsq[:])
                    o_mm(2)
                    o_mm(3)

                # epilogue: out = O * (NORM/rowsum); split DVE/ACT
                for pair in range(NSUB // 2):
                    osb = eout_pool.tile([P, 4 * D], BF16, tag="eout",
                                         name=f"eout{mb}_{pair}")
                    for k in range(2):
                        sub = 2 * pair + k
                        dst = osb[:, k * 2 * D:(k + 1) * 2 * D]
                        # mb=0: all muls on DVE so ACT stays exp-dedicated
                        # for the next m-block's prologue; last mb: split
                        # DVE/ACT to halve the tail.
                        if mb + 1 < MB or k == 0:
                            nc.vector.tensor_scalar_mul(
                                dst, o_ps[sub][:], recip[:, sub:sub + 1])
                        else:
                            nc.scalar.mul(dst, o_ps[sub][:],
                                          recip[:, sub:sub + 1])
                    col0 = (mb * NSUB + 2 * pair) * 2 * D
                    # alternate queues to shorten the tail
                    if pair % 2 == 0:
                        nc.sync.dma_start(out_d[:, col0:col0 + 4 * D], osb[:])
                    else:
                        nc.scalar.dma_start(out_d[:, col0:col0 + 4 * D], osb[:])

    nc.compile()
    return nc


def kernel(img, text, Wq, Wk):
    img = np.ascontiguousarray(img, dtype=np.float32)
    text = np.ascontiguousarray(text, dtype=np.float32)

    if "nc" not in _CACHE:
        _CACHE["nc"] = _build_nc()
    nc = _CACHE["nc"]

    # textT [256, 8192] f16 packed as [128, 16384]: the two d-halves of each
    # 1024-wide column group side by side (one tile + one DMA per group)
    tT = text.T.astype(np.float16)
    ttp = np.ascontiguousarray(
        tT.reshape(2, P, TTG, TTW).transpose(1, 2, 0, 3).reshape(P, 2 * N))
    # [img|text] bf16 rows, then two key-chunks (2*128 rows) packed side by
    # side so one 2KB-row DMA feeds two O-phase chunks.
    it_bf = np.concatenate([img, text], axis=1).astype(ml_dtypes.bfloat16)
    it2 = np.ascontiguousarray(
        it_bf.reshape(N // (2 * P), 2, P, 2 * D)
        .transpose(0, 2, 1, 3).reshape(N // 2, 4 * D))
    h16 = np.ascontiguousarray(
        (np.asarray(Wq, dtype=np.float32).T
         @ np.asarray(Wk, dtype=np.float32)).astype(np.float16))

    in_maps = []
    for c in range(N_CORES):
        in_maps.append({
            "it2_bf16": it2,
            "ttp_f16": ttp,
            "imgT_f16": np.ascontiguousarray(
                img[c * SLAB:(c + 1) * SLAB].T.astype(np.float16)),
            "H_f16": h16,
        })

    res = run_bass_kernel_spmd(nc, in_maps, core_ids=list(range(N_CORES)),
                               **_CACHE.get("run_kwargs", {}))
    _CACHE["last_results"] = res
    # unpack [128, 8*512] column-packed core outputs back to [1024, 512]
    out = np.concatenate([
        np.asarray(res.results[c]["out"], dtype=np.float32)
        .reshape(P, SLAB // P, 2 * D).transpose(1, 0, 2).reshape(SLAB, 2 * D)
        for c in range(N_CORES)], axis=0)
    return np.ascontiguousarray(out[:, :D]), np.ascontiguousarray(out[:, D:])


if __name__ == "__main__":
    rng = np.random.default_rng(0)
    img = rng.standard_normal((N, D), dtype=np.float32)
    text = rng.standard_normal((N, D), dtype=np.float32)
    sc = 1.0 / np.sqrt(D)
    Wq = rng.uniform(-sc, sc, (D, D)).astype(np.float32)
    Wk = rng.uniform(-sc, sc, (D, D)).astype(np.float32)
    oi, ot = kernel(img, text, Wq, Wk)
    print("out_img", oi.shape, oi.dtype, "out_text", ot.shape, ot.dtype)
